# revision 1
# baseline (speedup 1.0000x reference)
"""MoE (63 routed experts top-7 + 1 shared expert) Trainium2 kernel.

Strategy: expert-parallel sparse dispatch. The router (softmax + top-k,
~0.3% of FLOPs) runs on host; tokens are gathered expert-major into
fixed-capacity weight slots, which are distributed across 8 NeuronCores.
Each core runs an identical (SPMD) Bass program: for every slot, a
1280->1280 Linear + exact GELU + 1280->1280 Linear over 1024 tokens,
feature-major (features on partitions, tokens on the free dim) so weights
need no transpose and biases ride the activation unit's per-partition
bias port. Outputs are gathered and gate-weighted back on host in the
reference's exact accumulation order.
"""

import os
import sys
import math

sys.path.insert(0, "/opt/trn_rl_repo")

import numpy as np

D = 1280          # model dim
I = 1280          # expert inter dim
EXPERTS = 63      # routed experts
TOPK = 7          # routed top-k
CAP = 1024        # tokens per weight slot
CHUNK = 512       # tokens per matmul (fp32 moving-operand max)
KT = D // 128     # 10 contraction tiles
NCORES = 8

MM_DTYPE = os.environ.get("MM_DTYPE", "fp16")   # "fp16" | "f32r" | "bf16"

_PROGRAM_CACHE = {}


# ----------------------------------------------------------------- router

def _route(x2d, wr, br):
    """f32 softmax + top-k, matching jax.nn.softmax / jax.lax.top_k."""
    logits = (x2d @ wr + br).astype(np.float32)
    logits -= logits.max(-1, keepdims=True)
    np.exp(logits, out=logits)
    aff = logits / logits.sum(-1, keepdims=True)
    idx = np.argsort(-aff, axis=-1, kind="stable")[:, :TOPK]
    vals = np.take_along_axis(aff, idx, axis=-1)
    return idx.astype(np.int32), vals.astype(np.float32)


def _build_plan(T, idx):
    """Pack (token, expert) pairs expert-major into CAP-token pieces, plus
    the shared expert's T tokens, into 8 cores x S slots."""
    flat = idx.ravel()
    order = np.argsort(flat, kind="stable")          # expert-major slot order
    tok_of = (order // TOPK).astype(np.int64)
    counts = np.bincount(flat, minlength=EXPERTS)
    offs = np.concatenate([[0], np.cumsum(counts)])

    pieces = []  # (kind, expert, a, b)  [a:b) into the expert-major order
    for e in range(EXPERTS):
        a, b = int(offs[e]), int(offs[e + 1])
        while a < b:
            n = min(CAP, b - a)
            pieces.append(("r", e, a, a + n))
            a += n

    n_shared_min = math.ceil(T / CAP)
    S = max(1, math.ceil((len(pieces) + n_shared_min) / NCORES))
    n_shared = NCORES * S - len(pieces)
    # split T shared tokens near-evenly over n_shared pieces (each <= CAP)
    base, rem = divmod(T, n_shared)
    assert base + (1 if rem else 0) <= CAP
    t0 = 0
    for j in range(n_shared):
        n = base + (1 if j < rem else 0)
        pieces.append(("s", -1, t0, t0 + n))
        t0 += n
    assert t0 == T and len(pieces) == NCORES * S
    return pieces, S, order, tok_of


# ----------------------------------------------------------- device program

def _build_program(S, M):
    import concourse.bass as bass
    import concourse.mybir as mybir
    import concourse.tile as tile
    from concourse import bacc

    f32 = mybir.dt.float32
    in_dt = {"bf16": mybir.dt.bfloat16,
             "fp16": mybir.dt.float16,
             "f32r": mybir.dt.float32r}[MM_DTYPE]

    nc = bacc.Bacc("TRN2", target_bir_lowering=False, debug=False,
                   enable_asserts=False, num_devices=NCORES)
    xT = nc.dram_tensor("xT", [KT, 128, M], in_dt, kind="ExternalInput").ap()
    w1s = nc.dram_tensor("w1s", [S, KT, 128, KT, 128], in_dt, kind="ExternalInput").ap()
    w2s = nc.dram_tensor("w2s", [S, KT, 128, KT, 128], in_dt, kind="ExternalInput").ap()
    b1s = nc.dram_tensor("b1s", [S, 128, KT], f32, kind="ExternalInput").ap()
    b2s = nc.dram_tensor("b2s", [S, 128, KT], f32, kind="ExternalInput").ap()
    yT = nc.dram_tensor("yT", [KT, 128, M], f32, kind="ExternalOutput").ap()

    CPS = CAP // CHUNK  # chunks per slot
    Gelu = mybir.ActivationFunctionType.Gelu
    Ident = mybir.ActivationFunctionType.Identity

    def mm_ap(ap):
        return ap

    with tile.TileContext(nc) as tc:
        with (
            tc.tile_pool(name="xa", bufs=3) as xa,
            tc.tile_pool(name="w1p", bufs=4) as w1p,
            tc.tile_pool(name="w2p", bufs=4) as w2p,
            tc.tile_pool(name="hp", bufs=3) as hp,
            tc.tile_pool(name="yo", bufs=6) as yo,
            tc.tile_pool(name="bp", bufs=2) as bp,
            tc.tile_pool(name="ps", bufs=8, space="PSUM") as ps,
        ):
            for s in range(S):
                col0 = s * CAP
                b1t = bp.tile([128, KT], f32, tag="b1", name="b1t")
                nc.sync.dma_start(out=b1t[:, :], in_=b1s[s])
                b2t = bp.tile([128, KT], f32, tag="b2", name="b2t")
                nc.sync.dma_start(out=b2t[:, :], in_=b2s[s])

                xc = []
                for c in range(CPS):
                    xt = xa.tile([128, KT, CHUNK], in_dt, tag="x", name="xt")
                    for k in range(KT):
                        # SWDGE: keeps HWDGE free for the slot's weight loads
                        nc.gpsimd.dma_start(
                            out=xt[:, k, :],
                            in_=xT[k, :, col0 + c * CHUNK: col0 + (c + 1) * CHUNK])
                    xc.append(xt)

                hc = [hp.tile([128, KT, CHUNK], in_dt, tag="h", name=f"h{c}")
                      for c in range(CPS)]

                # layer 1: h = gelu(x @ w1 + b1), feature-major
                for io in range(KT):
                    w1t = w1p.tile([128, KT, 128], in_dt, tag="w1", name="w1t")
                    nc.sync.dma_start(out=w1t[:, :, :], in_=w1s[s, io])
                    for c in range(CPS):
                        pt = ps.tile([128, CHUNK], f32, tag="ps", name="pt")
                        for k in range(KT):
                            nc.tensor.matmul(pt[:, :], mm_ap(w1t[:, k, :]),
                                             mm_ap(xc[c][:, k, :]),
                                             start=(k == 0), stop=(k == KT - 1))
                        nc.scalar.activation(hc[c][:, io, :], pt[:, :], Gelu,
                                             bias=b1t[:, io:io + 1])

                # layer 2: y = h @ w2 + b2
                for io in range(KT):
                    w2t = w2p.tile([128, KT, 128], in_dt, tag="w2", name="w2t")
                    nc.sync.dma_start(out=w2t[:, :, :], in_=w2s[s, io])
                    for c in range(CPS):
                        pt = ps.tile([128, CHUNK], f32, tag="ps", name="pt")
                        for k in range(KT):
                            nc.tensor.matmul(pt[:, :], mm_ap(w2t[:, k, :]),
                                             mm_ap(hc[c][:, k, :]),
                                             start=(k == 0), stop=(k == KT - 1))
                        yt = yo.tile([128, CHUNK], f32, tag="y", name="yt")
                        nc.scalar.activation(yt[:, :], pt[:, :], Ident,
                                             bias=b2t[:, io:io + 1])
                        nc.sync.dma_start(
                            out=yT[io, :, col0 + c * CHUNK: col0 + (c + 1) * CHUNK],
                            in_=yt[:, :])
    nc.compile()
    return nc


def _get_program(S, M):
    key = (S, M, MM_DTYPE)
    if key not in _PROGRAM_CACHE:
        _PROGRAM_CACHE[key] = _build_program(S, M)
    return _PROGRAM_CACHE[key]


# ------------------------------------------------------------------ kernel

def _np_dt():
    import ml_dtypes
    return {"bf16": ml_dtypes.bfloat16, "fp16": np.float16,
            "f32r": np.float32}[MM_DTYPE]


def _arrange_w(w):
    """[D, I] -> [io, p, ko, c] so each (slot, io) block DMAs contiguously
    into an SBUF tile laid out [partition, ko, col]."""
    return np.ascontiguousarray(
        w.reshape(KT, 128, KT, 128).transpose(2, 1, 0, 3))


def kernel(x, sw1, sb1, sw2, sb2, rw1, rb1, rw2, rb2, wr, br, _trace=False):
    from concourse.bass_utils import run_bass_kernel_spmd

    x = np.asarray(x, dtype=np.float32)
    B, Sq, _ = x.shape
    T = B * Sq
    xf = np.ascontiguousarray(x.reshape(T, D))

    idx, vals = _route(xf, np.asarray(wr, np.float32), np.asarray(br, np.float32))
    pieces, S, order, tok_of = _build_plan(T, idx)
    M = S * CAP
    dt = _np_dt()

    rw1 = np.asarray(rw1, np.float32); rw2 = np.asarray(rw2, np.float32)
    rb1 = np.asarray(rb1, np.float32); rb2 = np.asarray(rb2, np.float32)
    sw1 = np.asarray(sw1, np.float32); sw2 = np.asarray(sw2, np.float32)
    sb1 = np.asarray(sb1, np.float32); sb2 = np.asarray(sb2, np.float32)

    # pre-arranged weights, cached per id of the weight arrays
    w1a = [_arrange_w(rw1[e]).astype(dt) for e in range(EXPERTS)]
    w2a = [_arrange_w(rw2[e]).astype(dt) for e in range(EXPERTS)]
    sw1a = _arrange_w(sw1).astype(dt)
    sw2a = _arrange_w(sw2).astype(dt)
    b1a = [np.ascontiguousarray(rb1[e].reshape(KT, 128).T) for e in range(EXPERTS)]
    b2a = [np.ascontiguousarray(rb2[e].reshape(KT, 128).T) for e in range(EXPERTS)]
    sb1a = np.ascontiguousarray(sb1.reshape(KT, 128).T)
    sb2a = np.ascontiguousarray(sb2.reshape(KT, 128).T)

    xfT = np.ascontiguousarray(xf.T)  # [D, T]
    tok_r = tok_of  # token of each expert-major (token,k) pair

    in_maps = []
    for core in range(NCORES):
        xT_core = np.zeros((D, M), dtype=dt)
        w1_core = np.zeros((S, KT, 128, KT, 128), dtype=dt)
        w2_core = np.zeros((S, KT, 128, KT, 128), dtype=dt)
        b1_core = np.zeros((S, 128, KT), dtype=np.float32)
        b2_core = np.zeros((S, 128, KT), dtype=np.float32)
        for j in range(S):
            kind, e, a, b = pieces[core * S + j]
            toks = tok_r[a:b] if kind == "r" else np.arange(a, b)
            xT_core[:, j * CAP: j * CAP + (b - a)] = xfT[:, toks]
            if kind == "r":
                w1_core[j] = w1a[e]; w2_core[j] = w2a[e]
                b1_core[j] = b1a[e]; b2_core[j] = b2a[e]
            else:
                w1_core[j] = sw1a; w2_core[j] = sw2a
                b1_core[j] = sb1a; b2_core[j] = sb2a
        in_maps.append({
            "xT": xT_core.reshape(KT, 128, M),
            "w1s": w1_core, "w2s": w2_core,
            "b1s": b1_core, "b2s": b2_core,
        })

    nc = _get_program(S, M)
    res = run_bass_kernel_spmd(nc, in_maps, core_ids=list(range(NCORES)),
                               trace=_trace)
    kernel.last_result = res

    TK = T * TOPK
    gated = np.empty((TK, D), dtype=np.float32)   # expert-major rows
    shared_out = np.empty((T, D), dtype=np.float32)
    for core in range(NCORES):
        Y = res.results[core]["yT"].reshape(D, M)
        for j in range(S):
            kind, e, a, b = pieces[core * S + j]
            block = Y[:, j * CAP: j * CAP + (b - a)].T  # [n, D]
            if kind == "r":
                gated[a:b] = block
            else:
                shared_out[a:b] = block

    g = vals.ravel()[order].astype(np.float32)
    gated *= g[:, None]
    ord2 = np.argsort(tok_of, kind="stable")      # token-major, expert asc
    routed = gated[ord2].reshape(T, TOPK, D).sum(axis=1, dtype=np.float32)

    out = shared_out + routed + xf
    return out.reshape(B, Sq, D).astype(np.float32)


kernel.last_result = None



# revision 2
# speedup vs baseline: 1.4952x; 1.4952x over previous
"""MoE (63 routed experts top-7 + 1 shared expert) Trainium2 kernel.

Strategy: expert-parallel sparse dispatch. The router (softmax + top-k,
~0.3% of FLOPs) runs on host; tokens are gathered expert-major into
fixed-capacity weight slots, which are distributed across 8 NeuronCores.
Each core runs an identical (SPMD) Bass program with R routed slots and
one shared-expert slot: per slot a 1280->1280 Linear + exact GELU +
1280->1280 Linear over 1024 tokens, feature-major (features on
partitions, tokens on the free dim) so weights need no transpose and
biases ride the activation unit's per-partition bias port.

Routed slots run in fp8e4 with DoubleRow perf mode (2 contraction slices
per PE pass, ~2x matmul throughput). Routed weights are pre-scaled by 64
so their 0.02-sigma values sit in fp8e4's normal range; the activation
scale port undoes it (psum/64 + bias). The shared expert stays fp16: its
output is unattenuated in the final sum while routed outputs are gated
by ~0.04-0.3, so fp8 noise there would dominate the error budget.
Outputs are gathered and gate-weighted back on host in the reference's
exact accumulation order.
"""

import math
import sys

sys.path.insert(0, "/opt/trn_rl_repo")

import numpy as np

D = 1280          # model dim
I = 1280          # expert inter dim
EXPERTS = 63      # routed experts
TOPK = 7          # routed top-k
CAP = 1024        # tokens per weight slot
CHUNK = 512       # tokens per matmul (PSUM bank = 512 fp32)
KT = D // 128     # 10 contraction tiles
KP = KT // 2      # 5 DoubleRow pairs
NCORES = 8
WSCALE = 64.0     # fp8 weight pre-scale (undone via activation scale)

_PROGRAM_CACHE = {}


# ----------------------------------------------------------------- router

def _route(x2d, wr, br):
    """f32 softmax + top-k, matching jax.nn.softmax / jax.lax.top_k."""
    logits = (x2d @ wr + br).astype(np.float32)
    logits -= logits.max(-1, keepdims=True)
    np.exp(logits, out=logits)
    aff = logits / logits.sum(-1, keepdims=True)
    idx = np.argsort(-aff, axis=-1, kind="stable")[:, :TOPK]
    vals = np.take_along_axis(aff, idx, axis=-1)
    return idx.astype(np.int32), vals.astype(np.float32)


def _build_plan(T, idx):
    """Pack (token, expert) pairs expert-major into CAP-token routed
    pieces (padded with empty pieces to a multiple of NCORES), plus
    NCORES shared pieces of exactly T/NCORES tokens."""
    flat = idx.ravel()
    order = np.argsort(flat, kind="stable")          # expert-major slot order
    tok_of = (order // TOPK).astype(np.int64)
    counts = np.bincount(flat, minlength=EXPERTS)
    offs = np.concatenate([[0], np.cumsum(counts)])

    rpieces = []  # (expert, a, b)  [a:b) into the expert-major order
    for e in range(EXPERTS):
        a, b = int(offs[e]), int(offs[e + 1])
        while a < b:
            n = min(CAP, b - a)
            rpieces.append((e, a, a + n))
            a += n

    R = math.ceil(len(rpieces) / NCORES)
    while len(rpieces) < NCORES * R:
        rpieces.append((0, 0, 0))                    # empty padding piece

    assert T % NCORES == 0 and T // NCORES <= CAP
    n = T // NCORES
    spieces = [(j * n, (j + 1) * n) for j in range(NCORES)]
    return rpieces, spieces, R, order, tok_of


# ----------------------------------------------------------- device program

def _build_program(R):
    import concourse.mybir as mybir
    import concourse.tile as tile
    from concourse import bacc

    f32 = mybir.dt.float32
    f16 = mybir.dt.float16
    f8 = mybir.dt.float8e4

    M8 = R * CAP
    M = (R + 1) * CAP

    nc = bacc.Bacc("TRN2", target_bir_lowering=False, debug=False,
                   enable_asserts=False, num_devices=NCORES)
    xT8 = nc.dram_tensor("xT8", [KT, 128, M8], f8, kind="ExternalInput").ap()
    xT16 = nc.dram_tensor("xT16", [KT, 128, CAP], f16, kind="ExternalInput").ap()
    w1s8 = nc.dram_tensor("w1s8", [R, KT, 128, KT, 128], f8, kind="ExternalInput").ap()
    w2s8 = nc.dram_tensor("w2s8", [R, KT, 128, KT, 128], f8, kind="ExternalInput").ap()
    w1s16 = nc.dram_tensor("w1s16", [KT, 128, KT, 128], f16, kind="ExternalInput").ap()
    w2s16 = nc.dram_tensor("w2s16", [KT, 128, KT, 128], f16, kind="ExternalInput").ap()
    b1s = nc.dram_tensor("b1s", [R + 1, 128, KT], f32, kind="ExternalInput").ap()
    b2s = nc.dram_tensor("b2s", [R + 1, 128, KT], f32, kind="ExternalInput").ap()
    yT = nc.dram_tensor("yT", [KT, 128, M], f32, kind="ExternalOutput").ap()

    CPS = CAP // CHUNK  # chunks per slot
    Gelu = mybir.ActivationFunctionType.Gelu
    Ident = mybir.ActivationFunctionType.Identity
    DR = mybir.MatmulPerfMode.DoubleRow

    with tile.TileContext(nc) as tc:
        with (
            tc.tile_pool(name="xa", bufs=3) as xa,
            tc.tile_pool(name="w1p", bufs=4) as w1p,
            tc.tile_pool(name="w2p", bufs=4) as w2p,
            tc.tile_pool(name="hp", bufs=3) as hp,
            tc.tile_pool(name="yo", bufs=6) as yo,
            tc.tile_pool(name="bp", bufs=2) as bp,
            tc.tile_pool(name="ps", bufs=8, space="PSUM") as ps,
        ):
            for s in range(R + 1):
                fp8 = s < R
                col0 = s * CAP
                in_dt = f8 if fp8 else f16
                sc = (1.0 / WSCALE) if fp8 else 1.0
                tg = "8" if fp8 else "16"

                b1t = bp.tile([128, KT], f32, tag="b1", name="b1t")
                nc.sync.dma_start(out=b1t[:, :], in_=b1s[s])
                b2t = bp.tile([128, KT], f32, tag="b2", name="b2t")
                nc.sync.dma_start(out=b2t[:, :], in_=b2s[s])

                xsrc = xT8 if fp8 else xT16
                xcol0 = col0 if fp8 else 0
                xc = []
                for c in range(CPS):
                    xt = xa.tile([128, KT, CHUNK], in_dt, tag="x" + tg, name="xt")
                    for k in range(KT):
                        # SWDGE: keeps HWDGE free for the slot's weight loads
                        nc.gpsimd.dma_start(
                            out=xt[:, k, :],
                            in_=xsrc[k, :, xcol0 + c * CHUNK: xcol0 + (c + 1) * CHUNK])
                    xc.append(xt)

                hc = [hp.tile([128, KT, CHUNK], in_dt, tag="h" + tg, name=f"h{c}")
                      for c in range(CPS)]

                # layer 1: h = gelu((x @ w1)/ws + b1), feature-major
                for io in range(KT):
                    w1t = w1p.tile([128, KT, 128], in_dt, tag="w1" + tg, name="w1t")
                    nc.sync.dma_start(out=w1t[:, :, :],
                                      in_=(w1s8[s, io] if fp8 else w1s16[io]))
                    for c in range(CPS):
                        pt = ps.tile([128, CHUNK], f32, tag="ps", name="pt")
                        if fp8:
                            for kp in range(KP):
                                nc.tensor.matmul(pt[:, :],
                                                 w1t[:, 2 * kp:2 * kp + 2, :],
                                                 xc[c][:, 2 * kp:2 * kp + 2, :],
                                                 start=(kp == 0), stop=(kp == KP - 1),
                                                 perf_mode=DR)
                        else:
                            for k in range(KT):
                                nc.tensor.matmul(pt[:, :], w1t[:, k, :],
                                                 xc[c][:, k, :],
                                                 start=(k == 0), stop=(k == KT - 1))
                        nc.scalar.activation(hc[c][:, io, :], pt[:, :], Gelu,
                                             bias=b1t[:, io:io + 1], scale=sc)

                # layer 2: y = (h @ w2)/ws + b2
                for io in range(KT):
                    w2t = w2p.tile([128, KT, 128], in_dt, tag="w2" + tg, name="w2t")
                    nc.sync.dma_start(out=w2t[:, :, :],
                                      in_=(w2s8[s, io] if fp8 else w2s16[io]))
                    for c in range(CPS):
                        pt = ps.tile([128, CHUNK], f32, tag="ps", name="pt")
                        if fp8:
                            for kp in range(KP):
                                nc.tensor.matmul(pt[:, :],
                                                 w2t[:, 2 * kp:2 * kp + 2, :],
                                                 hc[c][:, 2 * kp:2 * kp + 2, :],
                                                 start=(kp == 0), stop=(kp == KP - 1),
                                                 perf_mode=DR)
                        else:
                            for k in range(KT):
                                nc.tensor.matmul(pt[:, :], w2t[:, k, :],
                                                 hc[c][:, k, :],
                                                 start=(k == 0), stop=(k == KT - 1))
                        yt = yo.tile([128, CHUNK], f32, tag="y", name="yt")
                        nc.scalar.activation(yt[:, :], pt[:, :], Ident,
                                             bias=b2t[:, io:io + 1], scale=sc)
                        nc.sync.dma_start(
                            out=yT[io, :, col0 + c * CHUNK: col0 + (c + 1) * CHUNK],
                            in_=yt[:, :])
    nc.compile()
    return nc


def _get_program(R):
    if R not in _PROGRAM_CACHE:
        _PROGRAM_CACHE[R] = _build_program(R)
    return _PROGRAM_CACHE[R]


# ------------------------------------------------------------------ kernel

def _arrange_w(w):
    """[D, I] -> [io, p, ko, c] so each (slot, io) block DMAs contiguously
    into an SBUF tile laid out [partition, ko, col]."""
    return np.ascontiguousarray(
        w.reshape(KT, 128, KT, 128).transpose(2, 1, 0, 3))


def kernel(x, sw1, sb1, sw2, sb2, rw1, rb1, rw2, rb2, wr, br, _trace=False):
    from concourse.bass_utils import run_bass_kernel_spmd
    import ml_dtypes

    F8 = ml_dtypes.float8_e4m3

    x = np.asarray(x, dtype=np.float32)
    B, Sq, _ = x.shape
    T = B * Sq
    xf = np.ascontiguousarray(x.reshape(T, D))

    idx, vals = _route(xf, np.asarray(wr, np.float32), np.asarray(br, np.float32))
    rpieces, spieces, R, order, tok_of = _build_plan(T, idx)
    M = (R + 1) * CAP

    rw1 = np.asarray(rw1, np.float32); rw2 = np.asarray(rw2, np.float32)
    rb1 = np.asarray(rb1, np.float32); rb2 = np.asarray(rb2, np.float32)
    sw1 = np.asarray(sw1, np.float32); sw2 = np.asarray(sw2, np.float32)
    sb1 = np.asarray(sb1, np.float32); sb2 = np.asarray(sb2, np.float32)

    def q8(w):
        return np.clip(w * WSCALE, -240.0, 240.0).astype(F8)

    w1a = [_arrange_w(q8(rw1[e])) for e in range(EXPERTS)]
    w2a = [_arrange_w(q8(rw2[e])) for e in range(EXPERTS)]
    sw1a = _arrange_w(sw1).astype(np.float16)
    sw2a = _arrange_w(sw2).astype(np.float16)
    b1a = [np.ascontiguousarray(rb1[e].reshape(KT, 128).T) for e in range(EXPERTS)]
    b2a = [np.ascontiguousarray(rb2[e].reshape(KT, 128).T) for e in range(EXPERTS)]
    sb1a = np.ascontiguousarray(sb1.reshape(KT, 128).T)
    sb2a = np.ascontiguousarray(sb2.reshape(KT, 128).T)

    xfT = np.ascontiguousarray(xf.T)                          # [D, T] f32
    xq8T = np.ascontiguousarray(np.clip(xfT, -240.0, 240.0).astype(F8))

    in_maps = []
    for core in range(NCORES):
        xT8_core = np.zeros((D, R * CAP), dtype=F8)
        w1_8 = np.zeros((R, KT, 128, KT, 128), dtype=F8)
        w2_8 = np.zeros((R, KT, 128, KT, 128), dtype=F8)
        b1_c = np.zeros((R + 1, 128, KT), dtype=np.float32)
        b2_c = np.zeros((R + 1, 128, KT), dtype=np.float32)
        for j in range(R):
            e, a, b = rpieces[core * R + j]
            if b > a:
                xT8_core[:, j * CAP: j * CAP + (b - a)] = xq8T[:, tok_of[a:b]]
                w1_8[j] = w1a[e]; w2_8[j] = w2a[e]
                b1_c[j] = b1a[e]; b2_c[j] = b2a[e]
        a, b = spieces[core]
        xT16_core = np.ascontiguousarray(xfT[:, a:b]).astype(np.float16)
        b1_c[R] = sb1a; b2_c[R] = sb2a
        in_maps.append({
            "xT8": xT8_core.reshape(KT, 128, R * CAP),
            "xT16": xT16_core.reshape(KT, 128, CAP),
            "w1s8": w1_8, "w2s8": w2_8,
            "w1s16": sw1a, "w2s16": sw2a,
            "b1s": b1_c, "b2s": b2_c,
        })

    nc = _get_program(R)
    res = run_bass_kernel_spmd(nc, in_maps, core_ids=list(range(NCORES)),
                               trace=_trace)
    kernel.last_result = res

    TK = T * TOPK
    gated = np.empty((TK, D), dtype=np.float32)   # expert-major rows
    shared_out = np.empty((T, D), dtype=np.float32)
    for core in range(NCORES):
        Y = res.results[core]["yT"].reshape(D, M)
        for j in range(R):
            e, a, b = rpieces[core * R + j]
            if b > a:
                gated[a:b] = Y[:, j * CAP: j * CAP + (b - a)].T
        a, b = spieces[core]
        shared_out[a:b] = Y[:, R * CAP: R * CAP + (b - a)].T

    g = vals.ravel()[order].astype(np.float32)
    gated *= g[:, None]
    ord2 = np.argsort(tok_of, kind="stable")      # token-major, expert asc
    routed = gated[ord2].reshape(T, TOPK, D).sum(axis=1, dtype=np.float32)

    out = shared_out + routed + xf
    return out.reshape(B, Sq, D).astype(np.float32)


kernel.last_result = None


# revision 4
# speedup vs baseline: 1.5569x; 1.0413x over previous
"""MoE (63 routed experts top-7 + 1 shared expert) Trainium2 kernel.

Strategy: expert-parallel sparse dispatch. The router (softmax + top-k,
~0.3% of FLOPs) runs on host; tokens are gathered expert-major into
fixed-capacity weight slots, which are distributed across 8 NeuronCores.
Each core runs an identical (SPMD) Bass program with R routed slots and
one shared-expert slot: per slot a 1280->1280 Linear + exact GELU +
1280->1280 Linear over 1024 tokens, feature-major (features on
partitions, tokens on the free dim) so weights need no transpose and
biases ride the activation unit's per-partition bias port.

Routed slots run in fp8e4 with DoubleRow perf mode (2 contraction slices
per PE pass, ~2x matmul throughput). Routed weights are pre-scaled by 64
so their 0.02-sigma values sit in fp8e4's normal range; the activation
scale port undoes it (psum/64 + bias). The shared expert stays fp16: its
output is unattenuated in the final sum while routed outputs are gated
by ~0.04-0.3, so fp8 noise there would dominate the error budget.
Outputs are gathered and gate-weighted back on host in the reference's
exact accumulation order.
"""

import math
import sys

sys.path.insert(0, "/opt/trn_rl_repo")

import numpy as np

D = 1280          # model dim
I = 1280          # expert inter dim
EXPERTS = 63      # routed experts
TOPK = 7          # routed top-k
CAP = 1024        # tokens per weight slot
CHUNK = 512       # tokens per matmul (PSUM bank = 512 fp32)
KT = D // 128     # 10 contraction tiles
KP = KT // 2      # 5 DoubleRow pairs
NCORES = 8
WSCALE = 64.0     # fp8 weight pre-scale (undone via activation scale)

_PROGRAM_CACHE = {}


# ----------------------------------------------------------------- router

def _route(x2d, wr, br):
    """f32 softmax + top-k, matching jax.nn.softmax / jax.lax.top_k."""
    logits = (x2d @ wr + br).astype(np.float32)
    logits -= logits.max(-1, keepdims=True)
    np.exp(logits, out=logits)
    aff = logits / logits.sum(-1, keepdims=True)
    idx = np.argsort(-aff, axis=-1, kind="stable")[:, :TOPK]
    vals = np.take_along_axis(aff, idx, axis=-1)
    return idx.astype(np.int32), vals.astype(np.float32)


def _build_plan(T, idx):
    """Pack (token, expert) pairs expert-major into CAP-token routed
    pieces (padded with empty pieces to a multiple of NCORES), plus
    NCORES shared pieces of exactly T/NCORES tokens."""
    flat = idx.ravel()
    order = np.argsort(flat, kind="stable")          # expert-major slot order
    tok_of = (order // TOPK).astype(np.int64)
    counts = np.bincount(flat, minlength=EXPERTS)
    offs = np.concatenate([[0], np.cumsum(counts)])

    rpieces = []  # (expert, a, b)  [a:b) into the expert-major order
    for e in range(EXPERTS):
        a, b = int(offs[e]), int(offs[e + 1])
        while a < b:
            n = min(CAP, b - a)
            rpieces.append((e, a, a + n))
            a += n

    R = math.ceil(len(rpieces) / NCORES)
    while len(rpieces) < NCORES * R:
        rpieces.append((0, 0, 0))                    # empty padding piece

    assert T % NCORES == 0 and T // NCORES <= CAP
    n = T // NCORES
    spieces = [(j * n, (j + 1) * n) for j in range(NCORES)]
    return rpieces, spieces, R, order, tok_of


# ----------------------------------------------------------- device program

def _build_program(R):
    import concourse.mybir as mybir
    import concourse.tile as tile
    from concourse import bacc

    f32 = mybir.dt.float32
    f16 = mybir.dt.float16
    f8 = mybir.dt.float8e4

    M8 = R * CAP
    M = (R + 1) * CAP

    nc = bacc.Bacc("TRN2", target_bir_lowering=False, debug=False,
                   enable_asserts=False, num_devices=NCORES)
    xT8 = nc.dram_tensor("xT8", [KT, 128, M8], f8, kind="ExternalInput").ap()
    xT16 = nc.dram_tensor("xT16", [KT, 128, CAP], f16, kind="ExternalInput").ap()
    w1s8 = nc.dram_tensor("w1s8", [R, KT, 128, KT, 128], f8, kind="ExternalInput").ap()
    w2s8 = nc.dram_tensor("w2s8", [R, KT, 128, KT, 128], f8, kind="ExternalInput").ap()
    w1s16 = nc.dram_tensor("w1s16", [KT, 128, KT, 128], f16, kind="ExternalInput").ap()
    w2s16 = nc.dram_tensor("w2s16", [KT, 128, KT, 128], f16, kind="ExternalInput").ap()
    b1s = nc.dram_tensor("b1s", [R + 1, 128, KT], f32, kind="ExternalInput").ap()
    b2s = nc.dram_tensor("b2s", [R + 1, 128, KT], f32, kind="ExternalInput").ap()
    yT = nc.dram_tensor("yT", [KT, 128, M], f16, kind="ExternalOutput").ap()

    CPS = CAP // CHUNK  # chunks per slot
    Gelu = mybir.ActivationFunctionType.Gelu
    Ident = mybir.ActivationFunctionType.Identity
    DR = mybir.MatmulPerfMode.DoubleRow

    with tile.TileContext(nc) as tc:
        with (
            tc.tile_pool(name="xa", bufs=3) as xa,
            tc.tile_pool(name="w1p", bufs=4) as w1p,
            tc.tile_pool(name="w2p", bufs=4) as w2p,
            tc.tile_pool(name="hp", bufs=3) as hp,
            tc.tile_pool(name="yo", bufs=6) as yo,
            tc.tile_pool(name="bp", bufs=2) as bp,
            tc.tile_pool(name="ps", bufs=8, space="PSUM") as ps,
        ):
            for s in range(R + 1):
                fp8 = s < R
                col0 = s * CAP
                in_dt = f8 if fp8 else f16
                sc = (1.0 / WSCALE) if fp8 else 1.0
                tg = "8" if fp8 else "16"

                b1t = bp.tile([128, KT], f32, tag="b1", name="b1t")
                nc.sync.dma_start(out=b1t[:, :], in_=b1s[s])
                b2t = bp.tile([128, KT], f32, tag="b2", name="b2t")
                nc.sync.dma_start(out=b2t[:, :], in_=b2s[s])

                xsrc = xT8 if fp8 else xT16
                xcol0 = col0 if fp8 else 0
                xc = []
                for c in range(CPS):
                    xt = xa.tile([128, KT, CHUNK], in_dt, tag="x" + tg, name="xt")
                    for k in range(KT):
                        # SWDGE: keeps HWDGE free for the slot's weight loads
                        nc.gpsimd.dma_start(
                            out=xt[:, k, :],
                            in_=xsrc[k, :, xcol0 + c * CHUNK: xcol0 + (c + 1) * CHUNK])
                    xc.append(xt)

                hc = [hp.tile([128, KT, CHUNK], in_dt, tag="h" + tg, name=f"h{c}")
                      for c in range(CPS)]

                # layer 1: h = gelu((x @ w1)/ws + b1), feature-major.
                # k outer / chunk inner: each stationary weight block serves
                # CPS matmuls, keeping LDWEIGHTS at half the matmul time.
                for io in range(KT):
                    w1t = w1p.tile([128, KT, 128], in_dt, tag="w1" + tg, name="w1t")
                    nc.sync.dma_start(out=w1t[:, :, :],
                                      in_=(w1s8[s, io] if fp8 else w1s16[io]))
                    pt = [ps.tile([128, CHUNK], f32, tag="ps", name="pt")
                          for _ in range(CPS)]
                    if fp8:
                        for kp in range(KP):
                            for c in range(CPS):
                                nc.tensor.matmul(pt[c][:, :],
                                                 w1t[:, 2 * kp:2 * kp + 2, :],
                                                 xc[c][:, 2 * kp:2 * kp + 2, :],
                                                 start=(kp == 0), stop=(kp == KP - 1),
                                                 perf_mode=DR)
                    else:
                        for k in range(KT):
                            for c in range(CPS):
                                nc.tensor.matmul(pt[c][:, :], w1t[:, k, :],
                                                 xc[c][:, k, :],
                                                 start=(k == 0), stop=(k == KT - 1))
                    for c in range(CPS):
                        nc.scalar.activation(hc[c][:, io, :], pt[c][:, :], Gelu,
                                             bias=b1t[:, io:io + 1], scale=sc)

                # layer 2: y = (h @ w2)/ws + b2
                for io in range(KT):
                    w2t = w2p.tile([128, KT, 128], in_dt, tag="w2" + tg, name="w2t")
                    nc.sync.dma_start(out=w2t[:, :, :],
                                      in_=(w2s8[s, io] if fp8 else w2s16[io]))
                    pt = [ps.tile([128, CHUNK], f32, tag="ps", name="pt")
                          for _ in range(CPS)]
                    if fp8:
                        for kp in range(KP):
                            for c in range(CPS):
                                nc.tensor.matmul(pt[c][:, :],
                                                 w2t[:, 2 * kp:2 * kp + 2, :],
                                                 hc[c][:, 2 * kp:2 * kp + 2, :],
                                                 start=(kp == 0), stop=(kp == KP - 1),
                                                 perf_mode=DR)
                    else:
                        for k in range(KT):
                            for c in range(CPS):
                                nc.tensor.matmul(pt[c][:, :], w2t[:, k, :],
                                                 hc[c][:, k, :],
                                                 start=(k == 0), stop=(k == KT - 1))
                    for c in range(CPS):
                        yt = yo.tile([128, CHUNK], f16, tag="y", name="yt")
                        nc.scalar.activation(yt[:, :], pt[c][:, :], Ident,
                                             bias=b2t[:, io:io + 1], scale=sc)
                        nc.sync.dma_start(
                            out=yT[io, :, col0 + c * CHUNK: col0 + (c + 1) * CHUNK],
                            in_=yt[:, :])
    nc.compile()
    return nc


def _get_program(R):
    if R not in _PROGRAM_CACHE:
        _PROGRAM_CACHE[R] = _build_program(R)
    return _PROGRAM_CACHE[R]


# ------------------------------------------------------------------ kernel

def _arrange_w(w):
    """[D, I] -> [io, p, ko, c] so each (slot, io) block DMAs contiguously
    into an SBUF tile laid out [partition, ko, col]."""
    return np.ascontiguousarray(
        w.reshape(KT, 128, KT, 128).transpose(2, 1, 0, 3))


def kernel(x, sw1, sb1, sw2, sb2, rw1, rb1, rw2, rb2, wr, br, _trace=False):
    from concourse.bass_utils import run_bass_kernel_spmd
    import ml_dtypes

    F8 = ml_dtypes.float8_e4m3

    x = np.asarray(x, dtype=np.float32)
    B, Sq, _ = x.shape
    T = B * Sq
    xf = np.ascontiguousarray(x.reshape(T, D))

    idx, vals = _route(xf, np.asarray(wr, np.float32), np.asarray(br, np.float32))
    rpieces, spieces, R, order, tok_of = _build_plan(T, idx)
    M = (R + 1) * CAP

    rw1 = np.asarray(rw1, np.float32); rw2 = np.asarray(rw2, np.float32)
    rb1 = np.asarray(rb1, np.float32); rb2 = np.asarray(rb2, np.float32)
    sw1 = np.asarray(sw1, np.float32); sw2 = np.asarray(sw2, np.float32)
    sb1 = np.asarray(sb1, np.float32); sb2 = np.asarray(sb2, np.float32)

    def q8(w):
        return np.clip(w * WSCALE, -240.0, 240.0).astype(F8)

    w1a = [_arrange_w(q8(rw1[e])) for e in range(EXPERTS)]
    w2a = [_arrange_w(q8(rw2[e])) for e in range(EXPERTS)]
    sw1a = _arrange_w(sw1).astype(np.float16)
    sw2a = _arrange_w(sw2).astype(np.float16)
    b1a = [np.ascontiguousarray(rb1[e].reshape(KT, 128).T) for e in range(EXPERTS)]
    b2a = [np.ascontiguousarray(rb2[e].reshape(KT, 128).T) for e in range(EXPERTS)]
    sb1a = np.ascontiguousarray(sb1.reshape(KT, 128).T)
    sb2a = np.ascontiguousarray(sb2.reshape(KT, 128).T)

    xfT = np.ascontiguousarray(xf.T)                          # [D, T] f32
    xq8T = np.ascontiguousarray(np.clip(xfT, -240.0, 240.0).astype(F8))

    in_maps = []
    for core in range(NCORES):
        xT8_core = np.zeros((D, R * CAP), dtype=F8)
        w1_8 = np.zeros((R, KT, 128, KT, 128), dtype=F8)
        w2_8 = np.zeros((R, KT, 128, KT, 128), dtype=F8)
        b1_c = np.zeros((R + 1, 128, KT), dtype=np.float32)
        b2_c = np.zeros((R + 1, 128, KT), dtype=np.float32)
        for j in range(R):
            e, a, b = rpieces[core * R + j]
            if b > a:
                xT8_core[:, j * CAP: j * CAP + (b - a)] = xq8T[:, tok_of[a:b]]
                w1_8[j] = w1a[e]; w2_8[j] = w2a[e]
                b1_c[j] = b1a[e]; b2_c[j] = b2a[e]
        a, b = spieces[core]
        xT16_core = np.ascontiguousarray(xfT[:, a:b]).astype(np.float16)
        b1_c[R] = sb1a; b2_c[R] = sb2a
        in_maps.append({
            "xT8": xT8_core.reshape(KT, 128, R * CAP),
            "xT16": xT16_core.reshape(KT, 128, CAP),
            "w1s8": w1_8, "w2s8": w2_8,
            "w1s16": sw1a, "w2s16": sw2a,
            "b1s": b1_c, "b2s": b2_c,
        })

    nc = _get_program(R)
    res = run_bass_kernel_spmd(nc, in_maps, core_ids=list(range(NCORES)),
                               trace=_trace)
    kernel.last_result = res

    TK = T * TOPK
    gated = np.empty((TK, D), dtype=np.float32)   # expert-major rows
    shared_out = np.empty((T, D), dtype=np.float32)
    for core in range(NCORES):
        Y = res.results[core]["yT"].reshape(D, M)
        for j in range(R):
            e, a, b = rpieces[core * R + j]
            if b > a:
                gated[a:b] = Y[:, j * CAP: j * CAP + (b - a)].T
        a, b = spieces[core]
        shared_out[a:b] = Y[:, R * CAP: R * CAP + (b - a)].T

    g = vals.ravel()[order].astype(np.float32)
    gated *= g[:, None]
    ord2 = np.argsort(tok_of, kind="stable")      # token-major, expert asc
    routed = gated[ord2].reshape(T, TOPK, D).sum(axis=1, dtype=np.float32)

    out = shared_out + routed + xf
    return out.reshape(B, Sq, D).astype(np.float32)


kernel.last_result = None


# revision 6
# speedup vs baseline: 1.7242x; 1.1074x over previous
"""MoE (63 routed experts top-7 + 1 shared expert) Trainium2 kernel.

Strategy: expert-parallel sparse dispatch. The router (softmax + top-k,
~0.3% of FLOPs) runs on host; tokens are gathered expert-major into
fixed-capacity weight slots, which are distributed across 8 NeuronCores.
Each core runs an identical (SPMD) Bass program with R routed slots and
one shared-expert slot: per slot a 1280->1280 Linear + exact GELU +
1280->1280 Linear over 1024 tokens, feature-major (features on
partitions, tokens on the free dim) so weights need no transpose and
biases ride the activation unit's per-partition bias port.

Routed slots run in fp8e4 with DoubleRow perf mode (2 contraction slices
per PE pass, ~2x matmul throughput). Routed weights are pre-scaled by 64
so their 0.02-sigma values sit in fp8e4's normal range; the activation
scale port undoes it (psum/64 + bias). The shared expert stays fp16: its
output is unattenuated in the final sum while routed outputs are gated
by ~0.04-0.3, so fp8 noise there would dominate the error budget.
Outputs are gathered and gate-weighted back on host in the reference's
exact accumulation order.
"""

import math
import sys

sys.path.insert(0, "/opt/trn_rl_repo")

import numpy as np

D = 1280          # model dim
I = 1280          # expert inter dim
EXPERTS = 63      # routed experts
TOPK = 7          # routed top-k
CAP = 1024        # tokens per weight slot
CHUNK = 512       # tokens per matmul (PSUM bank = 512 fp32)
KT = D // 128     # 10 contraction tiles
KP = KT // 2      # 5 DoubleRow pairs
NCORES = 8
WSCALE = 64.0     # fp8 weight pre-scale (undone via activation scale)

_PROGRAM_CACHE = {}


# ----------------------------------------------------------------- router

def _route(x2d, wr, br):
    """f32 softmax + top-k, matching jax.nn.softmax / jax.lax.top_k."""
    logits = (x2d @ wr + br).astype(np.float32)
    logits -= logits.max(-1, keepdims=True)
    np.exp(logits, out=logits)
    aff = logits / logits.sum(-1, keepdims=True)
    idx = np.argsort(-aff, axis=-1, kind="stable")[:, :TOPK]
    vals = np.take_along_axis(aff, idx, axis=-1)
    return idx.astype(np.int32), vals.astype(np.float32)


def _build_plan(T, idx):
    """Pack (token, expert) pairs expert-major into CAP-token routed
    pieces (padded with empty pieces to a multiple of NCORES), plus
    NCORES shared pieces of exactly T/NCORES tokens."""
    flat = idx.ravel()
    order = np.argsort(flat, kind="stable")          # expert-major slot order
    tok_of = (order // TOPK).astype(np.int64)
    counts = np.bincount(flat, minlength=EXPERTS)
    offs = np.concatenate([[0], np.cumsum(counts)])

    rpieces = []  # (expert, a, b)  [a:b) into the expert-major order
    for e in range(EXPERTS):
        a, b = int(offs[e]), int(offs[e + 1])
        while a < b:
            n = min(CAP, b - a)
            rpieces.append((e, a, a + n))
            a += n

    R = math.ceil(len(rpieces) / NCORES)
    while len(rpieces) < NCORES * R:
        rpieces.append((0, 0, 0))                    # empty padding piece

    assert T % NCORES == 0 and T // NCORES <= CAP
    n = T // NCORES
    spieces = [(j * n, (j + 1) * n) for j in range(NCORES)]
    return rpieces, spieces, R, order, tok_of


# ----------------------------------------------------------- device program

def _build_program(R):
    import concourse.mybir as mybir
    import concourse.tile as tile
    from concourse import bacc

    f32 = mybir.dt.float32
    f16 = mybir.dt.float16
    f8 = mybir.dt.float8e4

    M8 = R * CAP
    M = (R + 1) * CAP

    nc = bacc.Bacc("TRN2", target_bir_lowering=False, debug=False,
                   enable_asserts=False, num_devices=NCORES)
    xT8 = nc.dram_tensor("xT8", [KT, 128, M8], f8, kind="ExternalInput").ap()
    xT16 = nc.dram_tensor("xT16", [KT, 128, CAP], f16, kind="ExternalInput").ap()
    w1s8 = nc.dram_tensor("w1s8", [R, KT, 128, KT, 128], f8, kind="ExternalInput").ap()
    w2s8 = nc.dram_tensor("w2s8", [R, KT, 128, KT, 128], f8, kind="ExternalInput").ap()
    w1s16 = nc.dram_tensor("w1s16", [KT, 128, KT, 128], f16, kind="ExternalInput").ap()
    w2s16 = nc.dram_tensor("w2s16", [KT, 128, KT, 128], f16, kind="ExternalInput").ap()
    b1s = nc.dram_tensor("b1s", [R + 1, 128, KT], f32, kind="ExternalInput").ap()
    b2s = nc.dram_tensor("b2s", [R + 1, 128, KT], f32, kind="ExternalInput").ap()
    yT = nc.dram_tensor("yT", [KT, 128, M], f16, kind="ExternalOutput").ap()

    CPS = CAP // CHUNK  # chunks per slot
    Gelu = mybir.ActivationFunctionType.Gelu
    Ident = mybir.ActivationFunctionType.Identity
    DR = mybir.MatmulPerfMode.DoubleRow

    with tile.TileContext(nc) as tc:
        with (
            tc.tile_pool(name="xa8", bufs=4) as xa8,
            tc.tile_pool(name="xa16", bufs=2) as xa16,
            tc.tile_pool(name="w1p8", bufs=8) as w1p8,
            tc.tile_pool(name="w2p8", bufs=8) as w2p8,
            tc.tile_pool(name="w1p16", bufs=2) as w1p16,
            tc.tile_pool(name="w2p16", bufs=2) as w2p16,
            tc.tile_pool(name="hp8", bufs=3) as hp8,
            tc.tile_pool(name="hp16", bufs=2) as hp16,
            tc.tile_pool(name="yo", bufs=6) as yo,
            tc.tile_pool(name="bp", bufs=2) as bp,
            tc.tile_pool(name="ps", bufs=8, space="PSUM") as ps,
        ):
            for s in range(R + 1):
                fp8 = s < R
                col0 = s * CAP
                in_dt = f8 if fp8 else f16
                sc = (1.0 / WSCALE) if fp8 else 1.0
                tg = "8" if fp8 else "16"
                xa = xa8 if fp8 else xa16
                w1p = w1p8 if fp8 else w1p16
                w2p = w2p8 if fp8 else w2p16
                hp = hp8 if fp8 else hp16

                b1t = bp.tile([128, KT], f32, tag="b1", name="b1t")
                nc.sync.dma_start(out=b1t[:, :], in_=b1s[s])
                b2t = bp.tile([128, KT], f32, tag="b2", name="b2t")
                nc.sync.dma_start(out=b2t[:, :], in_=b2s[s])

                xsrc = xT8 if fp8 else xT16
                xcol0 = col0 if fp8 else 0
                xc = []
                for c in range(CPS):
                    xt = xa.tile([128, KT, CHUNK], in_dt, tag="x" + tg, name="xt")
                    for k in range(KT):
                        # SWDGE: keeps HWDGE free for the slot's weight loads
                        nc.gpsimd.dma_start(
                            out=xt[:, k, :],
                            in_=xsrc[k, :, xcol0 + c * CHUNK: xcol0 + (c + 1) * CHUNK])
                    xc.append(xt)

                hc = [hp.tile([128, KT, CHUNK], in_dt, tag="h" + tg, name=f"h{c}")
                      for c in range(CPS)]

                # layer 1: h = gelu((x @ w1)/ws + b1), feature-major.
                # k outer / chunk inner: each stationary weight block serves
                # CPS matmuls, keeping LDWEIGHTS at half the matmul time.
                for io in range(KT):
                    w1t = w1p.tile([128, KT, 128], in_dt, tag="w1" + tg, name="w1t")
                    nc.sync.dma_start(out=w1t[:, :, :],
                                      in_=(w1s8[s, io] if fp8 else w1s16[io]))
                    pt = [ps.tile([128, CHUNK], f32, tag="ps", name="pt")
                          for _ in range(CPS)]
                    if fp8:
                        for kp in range(KP):
                            for c in range(CPS):
                                nc.tensor.matmul(pt[c][:, :],
                                                 w1t[:, 2 * kp:2 * kp + 2, :],
                                                 xc[c][:, 2 * kp:2 * kp + 2, :],
                                                 start=(kp == 0), stop=(kp == KP - 1),
                                                 perf_mode=DR)
                    else:
                        for k in range(KT):
                            for c in range(CPS):
                                nc.tensor.matmul(pt[c][:, :], w1t[:, k, :],
                                                 xc[c][:, k, :],
                                                 start=(k == 0), stop=(k == KT - 1))
                    for c in range(CPS):
                        nc.scalar.activation(hc[c][:, io, :], pt[c][:, :], Gelu,
                                             bias=b1t[:, io:io + 1], scale=sc)

                # layer 2: y = (h @ w2)/ws + b2
                for io in range(KT):
                    w2t = w2p.tile([128, KT, 128], in_dt, tag="w2" + tg, name="w2t")
                    nc.sync.dma_start(out=w2t[:, :, :],
                                      in_=(w2s8[s, io] if fp8 else w2s16[io]))
                    pt = [ps.tile([128, CHUNK], f32, tag="ps", name="pt")
                          for _ in range(CPS)]
                    if fp8:
                        for kp in range(KP):
                            for c in range(CPS):
                                nc.tensor.matmul(pt[c][:, :],
                                                 w2t[:, 2 * kp:2 * kp + 2, :],
                                                 hc[c][:, 2 * kp:2 * kp + 2, :],
                                                 start=(kp == 0), stop=(kp == KP - 1),
                                                 perf_mode=DR)
                    else:
                        for k in range(KT):
                            for c in range(CPS):
                                nc.tensor.matmul(pt[c][:, :], w2t[:, k, :],
                                                 hc[c][:, k, :],
                                                 start=(k == 0), stop=(k == KT - 1))
                    for c in range(CPS):
                        yt = yo.tile([128, CHUNK], f16, tag="y", name="yt")
                        nc.scalar.activation(yt[:, :], pt[c][:, :], Ident,
                                             bias=b2t[:, io:io + 1], scale=sc)
                        # store from the scalar engine's HWDGE queue so the
                        # sync queue stays dedicated to weight loads
                        nc.scalar.dma_start(
                            out=yT[io, :, col0 + c * CHUNK: col0 + (c + 1) * CHUNK],
                            in_=yt[:, :])
    nc.compile()
    return nc


def _get_program(R):
    if R not in _PROGRAM_CACHE:
        _PROGRAM_CACHE[R] = _build_program(R)
    return _PROGRAM_CACHE[R]


# ------------------------------------------------------------------ kernel

def _arrange_w(w):
    """[D, I] -> [io, p, ko, c] so each (slot, io) block DMAs contiguously
    into an SBUF tile laid out [partition, ko, col]."""
    return np.ascontiguousarray(
        w.reshape(KT, 128, KT, 128).transpose(2, 1, 0, 3))


def kernel(x, sw1, sb1, sw2, sb2, rw1, rb1, rw2, rb2, wr, br, _trace=False):
    from concourse.bass_utils import run_bass_kernel_spmd
    import ml_dtypes

    F8 = ml_dtypes.float8_e4m3

    x = np.asarray(x, dtype=np.float32)
    B, Sq, _ = x.shape
    T = B * Sq
    xf = np.ascontiguousarray(x.reshape(T, D))

    idx, vals = _route(xf, np.asarray(wr, np.float32), np.asarray(br, np.float32))
    rpieces, spieces, R, order, tok_of = _build_plan(T, idx)
    M = (R + 1) * CAP

    rw1 = np.asarray(rw1, np.float32); rw2 = np.asarray(rw2, np.float32)
    rb1 = np.asarray(rb1, np.float32); rb2 = np.asarray(rb2, np.float32)
    sw1 = np.asarray(sw1, np.float32); sw2 = np.asarray(sw2, np.float32)
    sb1 = np.asarray(sb1, np.float32); sb2 = np.asarray(sb2, np.float32)

    def q8(w):
        return np.clip(w * WSCALE, -240.0, 240.0).astype(F8)

    w1a = [_arrange_w(q8(rw1[e])) for e in range(EXPERTS)]
    w2a = [_arrange_w(q8(rw2[e])) for e in range(EXPERTS)]
    sw1a = _arrange_w(sw1).astype(np.float16)
    sw2a = _arrange_w(sw2).astype(np.float16)
    b1a = [np.ascontiguousarray(rb1[e].reshape(KT, 128).T) for e in range(EXPERTS)]
    b2a = [np.ascontiguousarray(rb2[e].reshape(KT, 128).T) for e in range(EXPERTS)]
    sb1a = np.ascontiguousarray(sb1.reshape(KT, 128).T)
    sb2a = np.ascontiguousarray(sb2.reshape(KT, 128).T)

    xfT = np.ascontiguousarray(xf.T)                          # [D, T] f32
    xq8T = np.ascontiguousarray(np.clip(xfT, -240.0, 240.0).astype(F8))

    in_maps = []
    for core in range(NCORES):
        xT8_core = np.zeros((D, R * CAP), dtype=F8)
        w1_8 = np.zeros((R, KT, 128, KT, 128), dtype=F8)
        w2_8 = np.zeros((R, KT, 128, KT, 128), dtype=F8)
        b1_c = np.zeros((R + 1, 128, KT), dtype=np.float32)
        b2_c = np.zeros((R + 1, 128, KT), dtype=np.float32)
        for j in range(R):
            e, a, b = rpieces[core * R + j]
            if b > a:
                xT8_core[:, j * CAP: j * CAP + (b - a)] = xq8T[:, tok_of[a:b]]
                w1_8[j] = w1a[e]; w2_8[j] = w2a[e]
                b1_c[j] = b1a[e]; b2_c[j] = b2a[e]
        a, b = spieces[core]
        xT16_core = np.ascontiguousarray(xfT[:, a:b]).astype(np.float16)
        b1_c[R] = sb1a; b2_c[R] = sb2a
        in_maps.append({
            "xT8": xT8_core.reshape(KT, 128, R * CAP),
            "xT16": xT16_core.reshape(KT, 128, CAP),
            "w1s8": w1_8, "w2s8": w2_8,
            "w1s16": sw1a, "w2s16": sw2a,
            "b1s": b1_c, "b2s": b2_c,
        })

    nc = _get_program(R)
    res = run_bass_kernel_spmd(nc, in_maps, core_ids=list(range(NCORES)),
                               trace=_trace)
    kernel.last_result = res

    TK = T * TOPK
    gated = np.empty((TK, D), dtype=np.float32)   # expert-major rows
    shared_out = np.empty((T, D), dtype=np.float32)
    for core in range(NCORES):
        Y = res.results[core]["yT"].reshape(D, M)
        for j in range(R):
            e, a, b = rpieces[core * R + j]
            if b > a:
                gated[a:b] = Y[:, j * CAP: j * CAP + (b - a)].T
        a, b = spieces[core]
        shared_out[a:b] = Y[:, R * CAP: R * CAP + (b - a)].T

    g = vals.ravel()[order].astype(np.float32)
    gated *= g[:, None]
    ord2 = np.argsort(tok_of, kind="stable")      # token-major, expert asc
    routed = gated[ord2].reshape(T, TOPK, D).sum(axis=1, dtype=np.float32)

    out = shared_out + routed + xf
    return out.reshape(B, Sq, D).astype(np.float32)


kernel.last_result = None
